# revision 20
# baseline (speedup 1.0000x reference)
"""Trainium2 Bass kernel for nn_Autoencoder_87205015978715 (topk_masking).

Reference model (per row of x):
    xe      = x + embed[layer_idx]
    h       = gelu(xe @ enc_w1 + enc_b1)          # (B, 1536), exact erf gelu
    logits  = h @ enc_w2 + enc_b2                 # (B, 256)
    vals,i  = top_k(logits, 4); w = softmax(vals)
    pts     = sum_k w_k * components[i_k]         # (B, 3)
    d       = gelu(pts @ dec_w1 + dec_b1)         # (B, 1536)
    decoded = d @ dec_w2 + dec_b2                 # (B, 2048)
    encoded = broadcast(components, (B, 256, 3))

Strategy: pure data parallel over 8 NeuronCores (2048 rows each).  Activations
are kept in transposed layout (feature dim on partitions, batch as the 512-wide
moving dim) so only x, logits and p need PE transposes.  The two large matmuls
(enc1, dec2) and dec1 run as a 3-matmul fp16 hi/lo decomposition
(a@b ~= ah@bh + ah@bl + al@bh, with the lo-residual of the small weights scaled
by 2^11 into fp16-normal range and the paired operand scaled by 2^-11), which
gives fp32-grade accuracy at 3 PE cycles/row instead of native fp32's 4
(and keeps the PE out of its fp32 half-clock mode).  enc2 uses the same
decomposition; measured logit error stays in the fp32 noise floor, which the
top-4 selection needs (min 4th-vs-5th logit gap for this seed is 6e-7).
Top-4 via the DVE Max8 instruction: the 4th-largest
logit is the mask threshold, then a masked-exp softmax over all 256 components
is mixed with a (256,3) matmul -- equivalent to the reference gather.
enc_w1 / enc_w2 / dec_w2 are streamed from HBM per 512-row block; the
embedding row is folded into the encoder bias on the host (b1_eff = enc_b1 +
embed[layer_idx] @ enc_w1).
"""

import os
import numpy as np
from contextlib import ExitStack

import concourse.bass as bass
from concourse import bacc
import concourse.tile as tile
import concourse.mybir as mybir
from concourse.bass_utils import run_bass_kernel_spmd
from concourse.masks import make_identity

F32 = mybir.dt.float32
F16 = mybir.dt.float16
AF = mybir.ActivationFunctionType
ALU = mybir.AluOpType

B, D, H, C = 16384, 2048, 1536, 256
NCORES = 8
BC = B // NCORES          # rows per core
BLK = 512                 # rows per block
P = 128
KD = D // P               # 16 contraction chunks for enc1
MH = H // P               # 12 feature chunks of H
MC = C // P               # 2 chunks of C
HH = H // 2               # enc_w1 column half streamed together
ND = D // 512              # 4 output column chunks of decoded
LS = 2.0 ** 11            # lo-residual scale (keeps residuals fp16-normal)


def _build(rows_per_core: int):
    nblk = rows_per_core // BLK
    rt_per_blk = BLK // P

    nc = bacc.Bacc("TRN2", target_bir_lowering=False, debug=False)
    x_d = nc.dram_tensor("x", [rows_per_core, D], F32, kind="ExternalInput")
    w1h_d = nc.dram_tensor("w1h", [D, H], F16, kind="ExternalInput")
    w1l_d = nc.dram_tensor("w1l", [D, H], F16, kind="ExternalInput")
    w2h_d = nc.dram_tensor("w2h", [H, C], F16, kind="ExternalInput")
    w2l_d = nc.dram_tensor("w2l", [H, C], F16, kind="ExternalInput")
    dw1h_d = nc.dram_tensor("dw1h", [P, H], F16, kind="ExternalInput")
    dw1l_d = nc.dram_tensor("dw1l", [P, H], F16, kind="ExternalInput")
    dw2h_d = nc.dram_tensor("dw2h", [H, D], F16, kind="ExternalInput")
    dw2l_d = nc.dram_tensor("dw2l", [H, D], F16, kind="ExternalInput")
    b1_d = nc.dram_tensor("b1", [P, MH], F32, kind="ExternalInput")
    b2_d = nc.dram_tensor("b2", [P, MC], F32, kind="ExternalInput")
    db1_d = nc.dram_tensor("db1", [P, MH], F32, kind="ExternalInput")
    db2_d = nc.dram_tensor("db2", [P, D], F32, kind="ExternalInput")
    comp_d = nc.dram_tensor("comp", [C, 3], F32, kind="ExternalInput")
    compb_d = nc.dram_tensor("compb", [P, C * 3], F32, kind="ExternalInput")
    enc_d = nc.dram_tensor("enc", [rows_per_core, C * 3], F32, kind="ExternalOutput")
    dec_d = nc.dram_tensor("dec", [rows_per_core, D], F32, kind="ExternalOutput")

    with tile.TileContext(nc) as tc, ExitStack() as ctx:
        const = ctx.enter_context(tc.tile_pool(name="const", bufs=1))
        xpool = ctx.enter_context(tc.tile_pool(name="xp", bufs=4))
        xtp = ctx.enter_context(tc.tile_pool(name="xtp", bufs=3))
        xsp = ctx.enter_context(tc.tile_pool(name="xsp", bufs=KD))
        hsp = ctx.enter_context(tc.tile_pool(name="hsp", bufs=MH))
        dsp = ctx.enter_context(tc.tile_pool(name="dsp", bufs=MH))
        w1s = ctx.enter_context(tc.tile_pool(name="w1s", bufs=4))
        w2rs = ctx.enter_context(tc.tile_pool(name="w2rs", bufs=3))
        w2s = ctx.enter_context(tc.tile_pool(name="w2s", bufs=6))
        tk = ctx.enter_context(tc.tile_pool(name="tk", bufs=2))
        outp = ctx.enter_context(tc.tile_pool(name="outp", bufs=2))
        pacc = ctx.enter_context(tc.tile_pool(name="pacc", bufs=6, space="PSUM"))
        ptr = ctx.enter_context(tc.tile_pool(name="ptr", bufs=2, space="PSUM"))

        ident = const.tile([P, P], F32, name="ident", tag="ident")
        make_identity(nc, ident[:])

        b1_sb = const.tile([P, MH], F32, name="b1", tag="b1")
        nc.sync.dma_start(b1_sb[:], b1_d[:])
        b2_sb = const.tile([P, MC], F32, name="b2", tag="b2")
        nc.sync.dma_start(b2_sb[:], b2_d[:])
        db1_sb = const.tile([P, MH], F32, name="db1", tag="db1")
        nc.sync.dma_start(db1_sb[:], db1_d[:])
        db2_sb = const.tile([P, D], F32, name="db2", tag="db2")
        nc.sync.dma_start(db2_sb[:], db2_d[:])
        compb_sb = const.tile([P, C * 3], F32, name="compb", tag="compb")
        nc.sync.dma_start(compb_sb[:], compb_d[:])
        dw1h_sb = const.tile([P, H], F16, name="dw1h", tag="dw1h")
        nc.sync.dma_start(dw1h_sb[:], dw1h_d[:])
        dw1l_sb = const.tile([P, H], F16, name="dw1l", tag="dw1l")
        nc.sync.dma_start(dw1l_sb[:], dw1l_d[:])
        comp_sb = []
        for kc in range(MC):
            t = const.tile([P, 3], F32, name=f"comp{kc}", tag=f"comp{kc}")
            nc.sync.dma_start(t[:], comp_d[kc * P:(kc + 1) * P, :])
            comp_sb.append(t)

        for blk in range(nblk):
            r0 = blk * BLK

            # ---- transpose x block into xT, split into fp16 hi/lo/scaled ----
            xts = []
            for kc in range(KD):
                xh = xsp.tile([P, BLK], F16, name="xh", tag="xh")
                xl = xsp.tile([P, BLK], F16, name="xl", tag="xl")
                xs = xsp.tile([P, BLK], F16, name="xs", tag="xs")
                xts.append((xh, xl, xs))
            for q in range(4):
                xrows = []
                for rt in range(rt_per_blk):
                    xrow = xpool.tile([P, 512], F32, name="x", tag="x")
                    nc.sync.dma_start(
                        xrow[:], x_d[r0 + rt * P: r0 + (rt + 1) * P,
                                     q * 512:(q + 1) * 512])
                    xrows.append(xrow)
                for rt in range(rt_per_blk):
                    xrow = xrows[rt]
                    for j in range(4):
                        kc = q * 4 + j
                        trp = ptr.tile([P, P], F32, name="tr", tag="tr")
                        nc.tensor.transpose(
                            trp[:], xrow[:, j * P:(j + 1) * P], ident[:])
                        xt32 = xtp.tile([P, P], F32, name="xt32", tag="s32x")
                        if kc % 2 == 0:
                            nc.vector.tensor_copy(xt32[:], trp[:])
                        else:
                            nc.scalar.copy(xt32[:], trp[:])
                        xh, xl, xs = xts[kc]
                        cs = slice(rt * P, (rt + 1) * P)
                        nc.vector.tensor_copy(xh[:, cs], xt32[:])
                        nc.gpsimd.tensor_tensor(
                            xl[:, cs], xt32[:], xh[:, cs], op=ALU.subtract)
                        nc.scalar.activation(
                            xs[:, cs], xt32[:], AF.Copy, scale=1.0 / LS)

            # ---- enc1 (fp16x3): hT[m] = gelu(W1[:,m].T @ xT + b1_eff[m]) ----
            hts = [None] * MH
            for mh in range(2):
                accs = [pacc.tile([P, BLK], F32, name="acc", tag="acc")
                        for _ in range(MH // 2)]
                for k in range(KD):
                    w1ht = w1s.tile([P, HH], F16, name="w1ht", tag="w1ht")
                    nc.sync.dma_start(
                        w1ht[:], w1h_d[k * P:(k + 1) * P, mh * HH:(mh + 1) * HH])
                    w1lt = w1s.tile([P, HH], F16, name="w1lt", tag="w1lt")
                    nc.sync.dma_start(
                        w1lt[:], w1l_d[k * P:(k + 1) * P, mh * HH:(mh + 1) * HH])
                    xh, xl, xs = xts[k]
                    for m6 in range(MH // 2):
                        ws = slice(m6 * P, (m6 + 1) * P)
                        nc.tensor.matmul(accs[m6][:], w1ht[:, ws], xh[:],
                                         start=(k == 0), stop=False)
                        nc.tensor.matmul(accs[m6][:], w1ht[:, ws], xl[:],
                                         start=False, stop=False)
                        nc.tensor.matmul(accs[m6][:], w1lt[:, ws], xs[:],
                                         start=False, stop=(k == KD - 1))
                for m6 in range(MH // 2):
                    m = mh * (MH // 2) + m6
                    ht32 = xtp.tile([P, BLK], F32, name="ht32", tag="s32")
                    nc.scalar.activation(ht32[:], accs[m6][:], AF.Gelu,
                                         bias=b1_sb[:, m:m + 1])
                    hh = hsp.tile([P, BLK], F16, name="hh", tag="hh")
                    nc.vector.tensor_copy(hh[:], ht32[:])
                    hl = hsp.tile([P, BLK], F16, name="hl", tag="hl")
                    nc.gpsimd.tensor_tensor(hl[:], ht32[:], hh[:],
                                            op=ALU.subtract)
                    hs = hsp.tile([P, BLK], F16, name="hs", tag="hs")
                    nc.scalar.activation(hs[:], ht32[:], AF.Copy,
                                         scale=1.0 / LS)
                    hts[m] = (hh, hl, hs)

            # ---- enc2 (fp16x3): logitsT[mc] = W2[:,mc].T @ hT + b2 ----
            lgaccs = [pacc.tile([P, BLK], F32, name="lga", tag="acc")
                      for _ in range(MC)]
            for k in range(MH):
                w2ht = w2rs.tile([P, C], F16, name="w2ht2", tag="w2ht2")
                nc.sync.dma_start(w2ht[:], w2h_d[k * P:(k + 1) * P, :])
                w2lt = w2rs.tile([P, C], F16, name="w2lt2", tag="w2lt2")
                nc.sync.dma_start(w2lt[:], w2l_d[k * P:(k + 1) * P, :])
                hh, hl, hs = hts[k]
                for mcc in range(MC):
                    ws = slice(mcc * P, (mcc + 1) * P)
                    nc.tensor.matmul(lgaccs[mcc][:], w2ht[:, ws], hh[:],
                                     start=(k == 0), stop=False)
                    nc.tensor.matmul(lgaccs[mcc][:], w2ht[:, ws], hl[:],
                                     start=False, stop=False)
                    nc.tensor.matmul(lgaccs[mcc][:], w2lt[:, ws], hs[:],
                                     start=False, stop=(k == MH - 1))
            lgts = []
            for mcc in range(MC):
                lt = tk.tile([P, BLK], F32, name=f"lgT{mcc}", tag=f"lgT{mcc}",
                             bufs=1)
                nc.scalar.activation(lt[:], lgaccs[mcc][:], AF.Identity,
                                     bias=b2_sb[:, mcc:mcc + 1])
                lgts.append(lt)

            # ---- top-4 masked softmax (per 128-row tile) ----
            pts_t = [tk.tile([P, BLK], F32, name=f"pT{mcc}", tag=f"pT{mcc}", bufs=1)
                     for mcc in range(MC)]
            for rt in range(rt_per_blk):
                lg = tk.tile([P, C], F32, name="lg", tag="lg")
                for mcc in range(MC):
                    trp = ptr.tile([P, P], F32, name="tr", tag="tr")
                    nc.tensor.transpose(
                        trp[:], lgts[mcc][:, rt * P:(rt + 1) * P], ident[:])
                    if mcc % 2 == 0:
                        nc.vector.tensor_copy(lg[:, mcc * P:(mcc + 1) * P], trp[:])
                    else:
                        nc.scalar.copy(lg[:, mcc * P:(mcc + 1) * P], trp[:])
                t8 = tk.tile([P, 8], F32, name="t8", tag="t8")
                nc.vector.max(out=t8[:], in_=lg[:])
                e = tk.tile([P, C], F32, name="e", tag="e")
                nc.scalar.activation(e[:], lg[:], AF.Exp)
                # em = (lg >= t4) * e ; s = sum(em)
                em = tk.tile([P, C], F32, name="em", tag="em")
                s = tk.tile([P, 1], F32, name="s", tag="s")
                nc.vector.scalar_tensor_tensor(
                    em[:], lg[:], t8[:, 3:4], e[:],
                    op0=ALU.is_ge, op1=ALU.mult, accum_out=s[:])
                r = tk.tile([P, 1], F32, name="r", tag="r")
                nc.vector.reciprocal(r[:], s[:])
                p = tk.tile([P, C], F32, name="p", tag="p")
                nc.vector.tensor_scalar(p[:], em[:], r[:, 0:1], None, op0=ALU.mult)
                for mcc in range(MC):
                    trp = ptr.tile([P, P], F32, name="tr", tag="tr")
                    nc.tensor.transpose(
                        trp[:], p[:, mcc * P:(mcc + 1) * P], ident[:])
                    if mcc % 2 == 0:
                        nc.vector.tensor_copy(
                            pts_t[mcc][:, rt * P:(rt + 1) * P], trp[:])
                    else:
                        nc.scalar.copy(
                            pts_t[mcc][:, rt * P:(rt + 1) * P], trp[:])

            # ---- mix: ptsT = components.T @ pT, replicated at 4 row groups ----
            acc = pacc.tile([P, BLK], F32, name="acc", tag="acc")
            for g in range(4):
                for kc in range(MC):
                    nc.tensor.matmul(acc[g * 32:g * 32 + 3, :],
                                     comp_sb[kc][:, 0:3], pts_t[kc][:],
                                     start=(kc == 0), stop=(kc == MC - 1),
                                     tile_position=(0, g * 32))
            ptst = tk.tile([P, BLK], F32, name="ptsT", tag="ptsT", bufs=1)
            nc.scalar.copy(ptst[:], acc[:])
            psh = tk.tile([P, BLK], F16, name="psh", tag="psh", bufs=1)
            nc.vector.tensor_copy(psh[:], ptst[:])
            psl = tk.tile([P, BLK], F16, name="psl", tag="psl", bufs=1)
            nc.vector.tensor_tensor(psl[:], ptst[:], psh[:], op=ALU.subtract)
            pss = tk.tile([P, BLK], F16, name="pss", tag="pss", bufs=1)
            nc.scalar.activation(pss[:], ptst[:], AF.Copy, scale=1.0 / LS)

            # ---- dec1 (fp16x3, 4 row-groups packed): dT[m] = gelu(...) ----
            dsplit = []
            for mg in range(MH // 4):
                gaccs = [pacc.tile([P, BLK], F32, name="acc", tag="acc")
                         for _ in range(4)]
                for prod in range(3):
                    for g in range(4):
                        m = mg * 4 + g
                        ws = slice(m * P, (m + 1) * P)
                        gp = slice(g * 32, g * 32 + 3)
                        if prod == 0:
                            lhs, rhs = dw1h_sb[gp, ws], psh[gp, :]
                        elif prod == 1:
                            lhs, rhs = dw1h_sb[gp, ws], psl[gp, :]
                        else:
                            lhs, rhs = dw1l_sb[gp, ws], pss[gp, :]
                        nc.tensor.matmul(gaccs[g][:], lhs, rhs,
                                         start=(prod == 0), stop=(prod == 2),
                                         tile_position=(g * 32, 0))
                for g in range(4):
                    m = mg * 4 + g
                    dt32 = xtp.tile([P, BLK], F32, name="dt32", tag="s32")
                    nc.scalar.activation(dt32[:], gaccs[g][:], AF.Gelu,
                                         bias=db1_sb[:, m:m + 1])
                    dh = dsp.tile([P, BLK], F16, name="dh", tag="dh")
                    nc.vector.tensor_copy(dh[:], dt32[:])
                    dl = dsp.tile([P, BLK], F16, name="dl", tag="dl")
                    nc.gpsimd.tensor_tensor(dl[:], dt32[:], dh[:],
                                            op=ALU.subtract)
                    ds = dsp.tile([P, BLK], F16, name="ds", tag="ds")
                    nc.scalar.activation(ds[:], dt32[:], AF.Copy, scale=1.0 / LS)
                    dsplit.append((dh, dl, ds))

            # ---- dec2 (fp16x3): decoded[b,n] = dT[:,b].T @ dw2[:,n] + db2 ----
            for n in range(ND):
                daccs = [pacc.tile([P, 512], F32, name="acc", tag="acc")
                         for _ in range(rt_per_blk)]
                for k in range(MH):
                    w2ht = w2s.tile([P, 512], F16, name="w2ht", tag="w2ht")
                    nc.sync.dma_start(
                        w2ht[:], dw2h_d[k * P:(k + 1) * P, n * 512:(n + 1) * 512])
                    w2lt = w2s.tile([P, 512], F16, name="w2lt", tag="w2lt")
                    nc.sync.dma_start(
                        w2lt[:], dw2l_d[k * P:(k + 1) * P, n * 512:(n + 1) * 512])
                    dh, dl, ds = dsplit[k]
                    for b in range(rt_per_blk):
                        bs = slice(b * P, (b + 1) * P)
                        nc.tensor.matmul(daccs[b][:], dh[:, bs], w2ht[:],
                                         start=(k == 0), stop=False)
                        nc.tensor.matmul(daccs[b][:], dl[:, bs], w2ht[:],
                                         start=False, stop=False)
                        nc.tensor.matmul(daccs[b][:], ds[:, bs], w2lt[:],
                                         start=False, stop=(k == MH - 1))
                for b in range(rt_per_blk):
                    o = outp.tile([P, 512], F32, name="out", tag="out")
                    nc.vector.tensor_add(o[:], daccs[b][:],
                                         db2_sb[:, n * 512:(n + 1) * 512])
                    nc.sync.dma_start(
                        dec_d[r0 + b * P: r0 + (b + 1) * P,
                              n * 512:(n + 1) * 512], o[:])

            # ---- encoded output: broadcast components ----
            for rt in range(rt_per_blk):
                nc.sync.dma_start(
                    enc_d[r0 + rt * P: r0 + (rt + 1) * P, :], compb_sb[:])

    nc.compile()
    return nc


_CACHE = {}


def _get_nc(rows_per_core: int):
    if rows_per_core not in _CACHE:
        _CACHE[rows_per_core] = _build(rows_per_core)
    return _CACHE[rows_per_core]


def _split16(w):
    hi = w.astype(np.float16)
    lo = ((w.astype(np.float64) - hi.astype(np.float64)) * LS).astype(np.float16)
    return np.ascontiguousarray(hi), np.ascontiguousarray(lo)


def prepare_inputs(x, layer_idx, embed, enc_w1, enc_b1, enc_w2, enc_b2,
                   components, dec_w1, dec_b1, dec_w2, dec_b2):
    x = np.ascontiguousarray(np.asarray(x, dtype=np.float32))
    embed = np.asarray(embed, dtype=np.float32)
    enc_w1 = np.ascontiguousarray(np.asarray(enc_w1, dtype=np.float32))
    enc_w2 = np.ascontiguousarray(np.asarray(enc_w2, dtype=np.float32))
    dec_w1 = np.ascontiguousarray(np.asarray(dec_w1, dtype=np.float32))
    dec_w2 = np.ascontiguousarray(np.asarray(dec_w2, dtype=np.float32))
    components = np.ascontiguousarray(np.asarray(components, dtype=np.float32))
    li = int(np.asarray(layer_idx))

    w1h, w1l = _split16(enc_w1)
    dw1h3, dw1l3 = _split16(dec_w1)
    # replicate dec_w1 at partition rows 0/32/64/96 for 4-way row-group packing
    dw1h = np.zeros((P, H), np.float16)
    dw1l = np.zeros((P, H), np.float16)
    for g in (0, 32, 64, 96):
        dw1h[g:g + 3] = dw1h3
        dw1l[g:g + 3] = dw1l3
    dw2h, dw2l = _split16(dec_w2)

    # Fold the embedding row into the encoder bias (in float64 on the host):
    # gelu((x+e) @ W1 + b1) == gelu(x @ W1 + (b1 + e @ W1)).
    b1_eff = (np.asarray(enc_b1, np.float64)
              + embed[li].astype(np.float64) @ enc_w1.astype(np.float64))
    w2h, w2l = _split16(enc_w2)
    shared = {
        "w1h": w1h, "w1l": w1l, "w2h": w2h, "w2l": w2l,
        "dw1h": dw1h, "dw1l": dw1l, "dw2h": dw2h, "dw2l": dw2l,
        "b1": np.ascontiguousarray(b1_eff.astype(np.float32).reshape(MH, P).T),
        "b2": np.ascontiguousarray(
            np.asarray(enc_b2, np.float32).reshape(MC, P).T),
        "db1": np.ascontiguousarray(
            np.asarray(dec_b1, np.float32).reshape(MH, P).T),
        "db2": np.ascontiguousarray(
            np.broadcast_to(np.asarray(dec_b2, np.float32), (P, D))),
        "comp": components,
        "compb": np.ascontiguousarray(
            np.broadcast_to(components.reshape(1, C * 3), (P, C * 3))),
    }
    return x, shared


def kernel(**inputs):
    x, shared = prepare_inputs(**inputs)
    nc = _get_nc(BC)
    in_maps = []
    for c in range(NCORES):
        m = dict(shared)
        m["x"] = np.ascontiguousarray(x[c * BC:(c + 1) * BC])
        in_maps.append(m)

    trace = bool(int(os.environ.get("KERNEL_TRACE", "0")))
    res = run_bass_kernel_spmd(nc, in_maps, core_ids=list(range(NCORES)),
                               trace=trace)
    if trace:
        kernel.last_exec_time_ns = res.exec_time_ns
        kernel.last_results = res

    decoded = np.concatenate([res.results[c]["dec"] for c in range(NCORES)], axis=0)
    encoded = np.concatenate([res.results[c]["enc"] for c in range(NCORES)],
                             axis=0).reshape(B, C, 3)
    return encoded, decoded


# revision 21
# speedup vs baseline: 1.0101x; 1.0101x over previous
"""Trainium2 Bass kernel for nn_Autoencoder_87205015978715 (topk_masking).

Reference model (per row of x):
    xe      = x + embed[layer_idx]
    h       = gelu(xe @ enc_w1 + enc_b1)          # (B, 1536), exact erf gelu
    logits  = h @ enc_w2 + enc_b2                 # (B, 256)
    vals,i  = top_k(logits, 4); w = softmax(vals)
    pts     = sum_k w_k * components[i_k]         # (B, 3)
    d       = gelu(pts @ dec_w1 + dec_b1)         # (B, 1536)
    decoded = d @ dec_w2 + dec_b2                 # (B, 2048)
    encoded = broadcast(components, (B, 256, 3))

Strategy: pure data parallel over 8 NeuronCores (2048 rows each).  Activations
are kept in transposed layout (feature dim on partitions, batch as the 512-wide
moving dim) so only x, logits and p need PE transposes.  The two large matmuls
(enc1, dec2) and dec1 run as a 3-matmul fp16 hi/lo decomposition
(a@b ~= ah@bh + ah@bl + al@bh, with the lo-residual of the small weights scaled
by 2^11 into fp16-normal range and the paired operand scaled by 2^-11), which
gives fp32-grade accuracy at 3 PE cycles/row instead of native fp32's 4
(and keeps the PE out of its fp32 half-clock mode).  enc2 uses the same
decomposition; measured logit error stays in the fp32 noise floor, which the
top-4 selection needs (min 4th-vs-5th logit gap for this seed is 6e-7).
Top-4 via the DVE Max8 instruction: the 4th-largest
logit is the mask threshold, then a masked-exp softmax over all 256 components
is mixed with a (256,3) matmul -- equivalent to the reference gather.
enc_w1 / enc_w2 / dec_w2 are streamed from HBM per 512-row block; the
embedding row is folded into the encoder bias on the host (b1_eff = enc_b1 +
embed[layer_idx] @ enc_w1).
"""

import os
import numpy as np
from contextlib import ExitStack

import concourse.bass as bass
from concourse import bacc
import concourse.tile as tile
import concourse.mybir as mybir
from concourse.bass_utils import run_bass_kernel_spmd
from concourse.masks import make_identity

F32 = mybir.dt.float32
F16 = mybir.dt.float16
AF = mybir.ActivationFunctionType
ALU = mybir.AluOpType

B, D, H, C = 16384, 2048, 1536, 256
NCORES = 8
BC = B // NCORES          # rows per core
BLK = 512                 # rows per block
P = 128
KD = D // P               # 16 contraction chunks for enc1
MH = H // P               # 12 feature chunks of H
MC = C // P               # 2 chunks of C
HH = H // 2               # enc_w1 column half streamed together
ND = D // 512              # 4 output column chunks of decoded
LS = 2.0 ** 11            # lo-residual scale (keeps residuals fp16-normal)


def _build(rows_per_core: int):
    nblk = rows_per_core // BLK
    rt_per_blk = BLK // P

    nc = bacc.Bacc("TRN2", target_bir_lowering=False, debug=False)
    x_d = nc.dram_tensor("x", [rows_per_core, D], F32, kind="ExternalInput")
    w1h_d = nc.dram_tensor("w1h", [D, H], F16, kind="ExternalInput")
    w1l_d = nc.dram_tensor("w1l", [D, H], F16, kind="ExternalInput")
    w2h_d = nc.dram_tensor("w2h", [H, C], F16, kind="ExternalInput")
    w2l_d = nc.dram_tensor("w2l", [H, C], F16, kind="ExternalInput")
    dw1h_d = nc.dram_tensor("dw1h", [P, H], F16, kind="ExternalInput")
    dw1l_d = nc.dram_tensor("dw1l", [P, H], F16, kind="ExternalInput")
    dw2h_d = nc.dram_tensor("dw2h", [H, D], F16, kind="ExternalInput")
    dw2l_d = nc.dram_tensor("dw2l", [H, D], F16, kind="ExternalInput")
    b1_d = nc.dram_tensor("b1", [P, MH], F32, kind="ExternalInput")
    b2_d = nc.dram_tensor("b2", [P, MC], F32, kind="ExternalInput")
    db1_d = nc.dram_tensor("db1", [P, MH], F32, kind="ExternalInput")
    db2_d = nc.dram_tensor("db2", [P, D], F32, kind="ExternalInput")
    comp_d = nc.dram_tensor("comp", [C, 3], F32, kind="ExternalInput")
    compb_d = nc.dram_tensor("compb", [P, C * 3], F32, kind="ExternalInput")
    enc_d = nc.dram_tensor("enc", [rows_per_core, C * 3], F32, kind="ExternalOutput")
    dec_d = nc.dram_tensor("dec", [rows_per_core, D], F32, kind="ExternalOutput")

    with tile.TileContext(nc) as tc, ExitStack() as ctx:
        const = ctx.enter_context(tc.tile_pool(name="const", bufs=1))
        xpool = ctx.enter_context(tc.tile_pool(name="xp", bufs=4))
        xtp = ctx.enter_context(tc.tile_pool(name="xtp", bufs=3))
        xsp = ctx.enter_context(tc.tile_pool(name="xsp", bufs=KD))
        hsp = ctx.enter_context(tc.tile_pool(name="hsp", bufs=MH))
        dsp = ctx.enter_context(tc.tile_pool(name="dsp", bufs=MH))
        w1s = ctx.enter_context(tc.tile_pool(name="w1s", bufs=4))
        w2rs = ctx.enter_context(tc.tile_pool(name="w2rs", bufs=3))
        w2s = ctx.enter_context(tc.tile_pool(name="w2s", bufs=6))
        tk = ctx.enter_context(tc.tile_pool(name="tk", bufs=2))
        outp = ctx.enter_context(tc.tile_pool(name="outp", bufs=2))
        pacc = ctx.enter_context(tc.tile_pool(name="pacc", bufs=6, space="PSUM"))
        ptr = ctx.enter_context(tc.tile_pool(name="ptr", bufs=2, space="PSUM"))

        ident = const.tile([P, P], F32, name="ident", tag="ident")
        make_identity(nc, ident[:])
        tbl = const.tile([P, 1], F32, name="tbl", tag="tbl")
        nc.gpsimd.memset(tbl[:], 0.0)

        b1_sb = const.tile([P, MH], F32, name="b1", tag="b1")
        nc.sync.dma_start(b1_sb[:], b1_d[:])
        b2_sb = const.tile([P, MC], F32, name="b2", tag="b2")
        nc.sync.dma_start(b2_sb[:], b2_d[:])
        db1_sb = const.tile([P, MH], F32, name="db1", tag="db1")
        nc.sync.dma_start(db1_sb[:], db1_d[:])
        db2_sb = const.tile([P, D], F32, name="db2", tag="db2")
        nc.sync.dma_start(db2_sb[:], db2_d[:])
        compb_sb = const.tile([P, C * 3], F32, name="compb", tag="compb")
        nc.sync.dma_start(compb_sb[:], compb_d[:])
        dw1h_sb = const.tile([P, H], F16, name="dw1h", tag="dw1h")
        nc.sync.dma_start(dw1h_sb[:], dw1h_d[:])
        dw1l_sb = const.tile([P, H], F16, name="dw1l", tag="dw1l")
        nc.sync.dma_start(dw1l_sb[:], dw1l_d[:])
        comp_sb = []
        for kc in range(MC):
            t = const.tile([P, 3], F32, name=f"comp{kc}", tag=f"comp{kc}")
            nc.sync.dma_start(t[:], comp_d[kc * P:(kc + 1) * P, :])
            comp_sb.append(t)

        for blk in range(nblk):
            r0 = blk * BLK

            # ---- transpose x block into xT, split into fp16 hi/lo/scaled ----
            xts = []
            for kc in range(KD):
                xh = xsp.tile([P, BLK], F16, name="xh", tag="xh")
                xl = xsp.tile([P, BLK], F16, name="xl", tag="xl")
                xs = xsp.tile([P, BLK], F16, name="xs", tag="xs")
                xts.append((xh, xl, xs))
            for q in range(4):
                xrows = []
                for rt in range(rt_per_blk):
                    xrow = xpool.tile([P, 512], F32, name="x", tag="x")
                    nc.sync.dma_start(
                        xrow[:], x_d[r0 + rt * P: r0 + (rt + 1) * P,
                                     q * 512:(q + 1) * 512])
                    xrows.append(xrow)
                for rt in range(rt_per_blk):
                    xrow = xrows[rt]
                    for j in range(4):
                        kc = q * 4 + j
                        trp = ptr.tile([P, P], F32, name="tr", tag="tr")
                        nc.tensor.transpose(
                            trp[:], xrow[:, j * P:(j + 1) * P], ident[:])
                        xt32 = xtp.tile([P, P], F32, name="xt32", tag="s32x")
                        if kc % 2 == 0:
                            nc.vector.tensor_copy(xt32[:], trp[:])
                        else:
                            nc.scalar.copy(xt32[:], trp[:])
                        xh, xl, xs = xts[kc]
                        cs = slice(rt * P, (rt + 1) * P)
                        nc.vector.tensor_copy(xh[:, cs], xt32[:])
                        nc.vector.tensor_tensor(
                            xl[:, cs], xt32[:], xh[:, cs], op=ALU.subtract)
                        nc.scalar.activation(
                            xs[:, cs], xt32[:], AF.Copy, scale=1.0 / LS)

            # ---- enc1 (fp16x3): hT[m] = gelu(W1[:,m].T @ xT + b1_eff[m]) ----
            hts = [None] * MH
            for mh in range(2):
                accs = [pacc.tile([P, BLK], F32, name="acc", tag="acc")
                        for _ in range(MH // 2)]
                for k in range(KD):
                    w1ht = w1s.tile([P, HH], F16, name="w1ht", tag="w1ht")
                    nc.sync.dma_start(
                        w1ht[:], w1h_d[k * P:(k + 1) * P, mh * HH:(mh + 1) * HH])
                    w1lt = w1s.tile([P, HH], F16, name="w1lt", tag="w1lt")
                    nc.sync.dma_start(
                        w1lt[:], w1l_d[k * P:(k + 1) * P, mh * HH:(mh + 1) * HH])
                    xh, xl, xs = xts[k]
                    for m6 in range(MH // 2):
                        ws = slice(m6 * P, (m6 + 1) * P)
                        nc.tensor.matmul(accs[m6][:], w1ht[:, ws], xh[:],
                                         start=(k == 0), stop=False)
                        nc.tensor.matmul(accs[m6][:], w1ht[:, ws], xl[:],
                                         start=False, stop=False)
                        nc.tensor.matmul(accs[m6][:], w1lt[:, ws], xs[:],
                                         start=False, stop=(k == KD - 1))
                for m6 in range(MH // 2):
                    m = mh * (MH // 2) + m6
                    ht32 = xtp.tile([P, BLK], F32, name="ht32", tag="s32")
                    nc.scalar.activation(ht32[:], accs[m6][:], AF.Gelu,
                                         bias=b1_sb[:, m:m + 1])
                    hh = hsp.tile([P, BLK], F16, name="hh", tag="hh")
                    nc.vector.tensor_copy(hh[:], ht32[:])
                    hl = hsp.tile([P, BLK], F16, name="hl", tag="hl")
                    nc.vector.tensor_tensor(hl[:], ht32[:], hh[:],
                                            op=ALU.subtract)
                    hs = hsp.tile([P, BLK], F16, name="hs", tag="hs")
                    nc.scalar.activation(hs[:], ht32[:], AF.Copy,
                                         scale=1.0 / LS)
                    hts[m] = (hh, hl, hs)

            # preload the exp table set off the top-k critical chain
            nc.scalar.activation(tbl[:], tbl[:], AF.Exp)

            # ---- enc2 (fp16x3): logitsT[mc] = W2[:,mc].T @ hT + b2 ----
            lgaccs = [pacc.tile([P, BLK], F32, name="lga", tag="acc")
                      for _ in range(MC)]
            for k in range(MH):
                w2ht = w2rs.tile([P, C], F16, name="w2ht2", tag="w2ht2")
                nc.sync.dma_start(w2ht[:], w2h_d[k * P:(k + 1) * P, :])
                w2lt = w2rs.tile([P, C], F16, name="w2lt2", tag="w2lt2")
                nc.sync.dma_start(w2lt[:], w2l_d[k * P:(k + 1) * P, :])
                hh, hl, hs = hts[k]
                for mcc in range(MC):
                    ws = slice(mcc * P, (mcc + 1) * P)
                    nc.tensor.matmul(lgaccs[mcc][:], w2ht[:, ws], hh[:],
                                     start=(k == 0), stop=False)
                    nc.tensor.matmul(lgaccs[mcc][:], w2ht[:, ws], hl[:],
                                     start=False, stop=False)
                    nc.tensor.matmul(lgaccs[mcc][:], w2lt[:, ws], hs[:],
                                     start=False, stop=(k == MH - 1))
            lgts = []
            for mcc in range(MC):
                lt = tk.tile([P, BLK], F32, name=f"lgT{mcc}", tag=f"lgT{mcc}",
                             bufs=1)
                nc.scalar.activation(lt[:], lgaccs[mcc][:], AF.Identity,
                                     bias=b2_sb[:, mcc:mcc + 1])
                lgts.append(lt)

            # ---- top-4 masked softmax (per 128-row tile) ----
            pts_t = [tk.tile([P, BLK], F32, name=f"pT{mcc}", tag=f"pT{mcc}", bufs=1)
                     for mcc in range(MC)]
            for rt in range(rt_per_blk):
                lg = tk.tile([P, C], F32, name="lg", tag="lg")
                for mcc in range(MC):
                    trp = ptr.tile([P, P], F32, name="tr", tag="tr")
                    nc.tensor.transpose(
                        trp[:], lgts[mcc][:, rt * P:(rt + 1) * P], ident[:])
                    if mcc % 2 == 0:
                        nc.vector.tensor_copy(lg[:, mcc * P:(mcc + 1) * P], trp[:])
                    else:
                        nc.scalar.copy(lg[:, mcc * P:(mcc + 1) * P], trp[:])
                t8 = tk.tile([P, 8], F32, name="t8", tag="t8")
                nc.vector.max(out=t8[:], in_=lg[:])
                e = tk.tile([P, C], F32, name="e", tag="e")
                nc.scalar.activation(e[:], lg[:], AF.Exp)
                # em = (lg >= t4) * e ; s = sum(em)
                em = tk.tile([P, C], F32, name="em", tag="em")
                s = tk.tile([P, 1], F32, name="s", tag="s")
                nc.vector.scalar_tensor_tensor(
                    em[:], lg[:], t8[:, 3:4], e[:],
                    op0=ALU.is_ge, op1=ALU.mult, accum_out=s[:])
                r = tk.tile([P, 1], F32, name="r", tag="r")
                nc.vector.reciprocal(r[:], s[:])
                p = tk.tile([P, C], F32, name="p", tag="p")
                nc.vector.tensor_scalar(p[:], em[:], r[:, 0:1], None, op0=ALU.mult)
                for mcc in range(MC):
                    trp = ptr.tile([P, P], F32, name="tr", tag="tr")
                    nc.tensor.transpose(
                        trp[:], p[:, mcc * P:(mcc + 1) * P], ident[:])
                    if mcc % 2 == 0:
                        nc.vector.tensor_copy(
                            pts_t[mcc][:, rt * P:(rt + 1) * P], trp[:])
                    else:
                        nc.scalar.copy(
                            pts_t[mcc][:, rt * P:(rt + 1) * P], trp[:])

            # preload the gelu table set off the dec1 critical chain
            nc.scalar.activation(tbl[:], tbl[:], AF.Gelu)

            # ---- mix: ptsT = components.T @ pT, replicated at 4 row groups ----
            acc = pacc.tile([P, BLK], F32, name="acc", tag="acc")
            for g in range(4):
                for kc in range(MC):
                    nc.tensor.matmul(acc[g * 32:g * 32 + 3, :],
                                     comp_sb[kc][:, 0:3], pts_t[kc][:],
                                     start=(kc == 0), stop=(kc == MC - 1),
                                     tile_position=(0, g * 32))
            ptst = tk.tile([P, BLK], F32, name="ptsT", tag="ptsT", bufs=1)
            nc.scalar.copy(ptst[:], acc[:])
            psh = tk.tile([P, BLK], F16, name="psh", tag="psh", bufs=1)
            nc.vector.tensor_copy(psh[:], ptst[:])
            psl = tk.tile([P, BLK], F16, name="psl", tag="psl", bufs=1)
            nc.vector.tensor_tensor(psl[:], ptst[:], psh[:], op=ALU.subtract)
            pss = tk.tile([P, BLK], F16, name="pss", tag="pss", bufs=1)
            nc.scalar.activation(pss[:], ptst[:], AF.Copy, scale=1.0 / LS)

            # ---- dec1 (fp16x3, 4 row-groups packed): dT[m] = gelu(...) ----
            dsplit = []
            for mg in range(MH // 4):
                gaccs = [pacc.tile([P, BLK], F32, name="acc", tag="acc")
                         for _ in range(4)]
                for prod in range(3):
                    for g in range(4):
                        m = mg * 4 + g
                        ws = slice(m * P, (m + 1) * P)
                        gp = slice(g * 32, g * 32 + 3)
                        if prod == 0:
                            lhs, rhs = dw1h_sb[gp, ws], psh[gp, :]
                        elif prod == 1:
                            lhs, rhs = dw1h_sb[gp, ws], psl[gp, :]
                        else:
                            lhs, rhs = dw1l_sb[gp, ws], pss[gp, :]
                        nc.tensor.matmul(gaccs[g][:], lhs, rhs,
                                         start=(prod == 0), stop=(prod == 2),
                                         tile_position=(g * 32, 0))
                for g in range(4):
                    m = mg * 4 + g
                    dt32 = xtp.tile([P, BLK], F32, name="dt32", tag="s32")
                    nc.scalar.activation(dt32[:], gaccs[g][:], AF.Gelu,
                                         bias=db1_sb[:, m:m + 1])
                    dh = dsp.tile([P, BLK], F16, name="dh", tag="dh")
                    nc.vector.tensor_copy(dh[:], dt32[:])
                    dl = dsp.tile([P, BLK], F16, name="dl", tag="dl")
                    nc.vector.tensor_tensor(dl[:], dt32[:], dh[:],
                                            op=ALU.subtract)
                    ds = dsp.tile([P, BLK], F16, name="ds", tag="ds")
                    nc.scalar.activation(ds[:], dt32[:], AF.Copy, scale=1.0 / LS)
                    dsplit.append((dh, dl, ds))

            # ---- dec2 (fp16x3): decoded[b,n] = dT[:,b].T @ dw2[:,n] + db2 ----
            for n in range(ND):
                daccs = [pacc.tile([P, 512], F32, name="acc", tag="acc")
                         for _ in range(rt_per_blk)]
                for k in range(MH):
                    w2ht = w2s.tile([P, 512], F16, name="w2ht", tag="w2ht")
                    nc.sync.dma_start(
                        w2ht[:], dw2h_d[k * P:(k + 1) * P, n * 512:(n + 1) * 512])
                    w2lt = w2s.tile([P, 512], F16, name="w2lt", tag="w2lt")
                    nc.sync.dma_start(
                        w2lt[:], dw2l_d[k * P:(k + 1) * P, n * 512:(n + 1) * 512])
                    dh, dl, ds = dsplit[k]
                    for b in range(rt_per_blk):
                        bs = slice(b * P, (b + 1) * P)
                        nc.tensor.matmul(daccs[b][:], dh[:, bs], w2ht[:],
                                         start=(k == 0), stop=False)
                        nc.tensor.matmul(daccs[b][:], dl[:, bs], w2ht[:],
                                         start=False, stop=False)
                        nc.tensor.matmul(daccs[b][:], ds[:, bs], w2lt[:],
                                         start=False, stop=(k == MH - 1))
                for b in range(rt_per_blk):
                    o = outp.tile([P, 512], F32, name="out", tag="out")
                    nc.vector.tensor_add(o[:], daccs[b][:],
                                         db2_sb[:, n * 512:(n + 1) * 512])
                    nc.sync.dma_start(
                        dec_d[r0 + b * P: r0 + (b + 1) * P,
                              n * 512:(n + 1) * 512], o[:])

            # ---- encoded output: broadcast components ----
            for rt in range(rt_per_blk):
                nc.sync.dma_start(
                    enc_d[r0 + rt * P: r0 + (rt + 1) * P, :], compb_sb[:])

    nc.compile()
    return nc


_CACHE = {}


def _get_nc(rows_per_core: int):
    if rows_per_core not in _CACHE:
        _CACHE[rows_per_core] = _build(rows_per_core)
    return _CACHE[rows_per_core]


def _split16(w):
    hi = w.astype(np.float16)
    lo = ((w.astype(np.float64) - hi.astype(np.float64)) * LS).astype(np.float16)
    return np.ascontiguousarray(hi), np.ascontiguousarray(lo)


def prepare_inputs(x, layer_idx, embed, enc_w1, enc_b1, enc_w2, enc_b2,
                   components, dec_w1, dec_b1, dec_w2, dec_b2):
    x = np.ascontiguousarray(np.asarray(x, dtype=np.float32))
    embed = np.asarray(embed, dtype=np.float32)
    enc_w1 = np.ascontiguousarray(np.asarray(enc_w1, dtype=np.float32))
    enc_w2 = np.ascontiguousarray(np.asarray(enc_w2, dtype=np.float32))
    dec_w1 = np.ascontiguousarray(np.asarray(dec_w1, dtype=np.float32))
    dec_w2 = np.ascontiguousarray(np.asarray(dec_w2, dtype=np.float32))
    components = np.ascontiguousarray(np.asarray(components, dtype=np.float32))
    li = int(np.asarray(layer_idx))

    w1h, w1l = _split16(enc_w1)
    dw1h3, dw1l3 = _split16(dec_w1)
    # replicate dec_w1 at partition rows 0/32/64/96 for 4-way row-group packing
    dw1h = np.zeros((P, H), np.float16)
    dw1l = np.zeros((P, H), np.float16)
    for g in (0, 32, 64, 96):
        dw1h[g:g + 3] = dw1h3
        dw1l[g:g + 3] = dw1l3
    dw2h, dw2l = _split16(dec_w2)

    # Fold the embedding row into the encoder bias (in float64 on the host):
    # gelu((x+e) @ W1 + b1) == gelu(x @ W1 + (b1 + e @ W1)).
    b1_eff = (np.asarray(enc_b1, np.float64)
              + embed[li].astype(np.float64) @ enc_w1.astype(np.float64))
    w2h, w2l = _split16(enc_w2)
    shared = {
        "w1h": w1h, "w1l": w1l, "w2h": w2h, "w2l": w2l,
        "dw1h": dw1h, "dw1l": dw1l, "dw2h": dw2h, "dw2l": dw2l,
        "b1": np.ascontiguousarray(b1_eff.astype(np.float32).reshape(MH, P).T),
        "b2": np.ascontiguousarray(
            np.asarray(enc_b2, np.float32).reshape(MC, P).T),
        "db1": np.ascontiguousarray(
            np.asarray(dec_b1, np.float32).reshape(MH, P).T),
        "db2": np.ascontiguousarray(
            np.broadcast_to(np.asarray(dec_b2, np.float32), (P, D))),
        "comp": components,
        "compb": np.ascontiguousarray(
            np.broadcast_to(components.reshape(1, C * 3), (P, C * 3))),
    }
    return x, shared


def kernel(**inputs):
    x, shared = prepare_inputs(**inputs)
    nc = _get_nc(BC)
    in_maps = []
    for c in range(NCORES):
        m = dict(shared)
        m["x"] = np.ascontiguousarray(x[c * BC:(c + 1) * BC])
        in_maps.append(m)

    trace = bool(int(os.environ.get("KERNEL_TRACE", "0")))
    res = run_bass_kernel_spmd(nc, in_maps, core_ids=list(range(NCORES)),
                               trace=trace)
    if trace:
        kernel.last_exec_time_ns = res.exec_time_ns
        kernel.last_results = res

    decoded = np.concatenate([res.results[c]["dec"] for c in range(NCORES)], axis=0)
    encoded = np.concatenate([res.results[c]["enc"] for c in range(NCORES)],
                             axis=0).reshape(B, C, 3)
    return encoded, decoded


# revision 22
# speedup vs baseline: 1.0329x; 1.0225x over previous
"""Trainium2 Bass kernel for nn_Autoencoder_87205015978715 (topk_masking).

Reference model (per row of x):
    xe      = x + embed[layer_idx]
    h       = gelu(xe @ enc_w1 + enc_b1)          # (B, 1536), exact erf gelu
    logits  = h @ enc_w2 + enc_b2                 # (B, 256)
    vals,i  = top_k(logits, 4); w = softmax(vals)
    pts     = sum_k w_k * components[i_k]         # (B, 3)
    d       = gelu(pts @ dec_w1 + dec_b1)         # (B, 1536)
    decoded = d @ dec_w2 + dec_b2                 # (B, 2048)
    encoded = broadcast(components, (B, 256, 3))

Strategy: pure data parallel over 8 NeuronCores (2048 rows each).  Activations
are kept in transposed layout (feature dim on partitions, batch as the 512-wide
moving dim) so only x, logits and p need PE transposes.  The two large matmuls
(enc1, dec2) and dec1 run as a 3-matmul fp16 hi/lo decomposition
(a@b ~= ah@bh + ah@bl + al@bh, with the lo-residual of the small weights scaled
by 2^11 into fp16-normal range and the paired operand scaled by 2^-11), which
gives fp32-grade accuracy at 3 PE cycles/row instead of native fp32's 4
(and keeps the PE out of its fp32 half-clock mode).  enc2 uses the same
decomposition; measured logit error stays in the fp32 noise floor, which the
top-4 selection needs (min 4th-vs-5th logit gap for this seed is 6e-7).
Top-4 via the DVE Max8 instruction: the 4th-largest
logit is the mask threshold, then a masked-exp softmax over all 256 components
is mixed with a (256,3) matmul -- equivalent to the reference gather.
enc_w1 / enc_w2 / dec_w2 are streamed from HBM per 512-row block; the
embedding row is folded into the encoder bias on the host (b1_eff = enc_b1 +
embed[layer_idx] @ enc_w1).
"""

import os
import numpy as np
from contextlib import ExitStack

import concourse.bass as bass
from concourse import bacc
import concourse.tile as tile
import concourse.mybir as mybir
from concourse.bass_utils import run_bass_kernel_spmd
from concourse.masks import make_identity

F32 = mybir.dt.float32
F16 = mybir.dt.float16
AF = mybir.ActivationFunctionType
ALU = mybir.AluOpType

B, D, H, C = 16384, 2048, 1536, 256
NCORES = 8
BC = B // NCORES          # rows per core
BLK = 512                 # rows per block
P = 128
KD = D // P               # 16 contraction chunks for enc1
MH = H // P               # 12 feature chunks of H
MC = C // P               # 2 chunks of C
HH = H // 2               # enc_w1 column half streamed together
ND = D // 512              # 4 output column chunks of decoded
LS = 2.0 ** 11            # lo-residual scale (keeps residuals fp16-normal)


def _build(rows_per_core: int):
    nblk = rows_per_core // BLK
    rt_per_blk = BLK // P

    nc = bacc.Bacc("TRN2", target_bir_lowering=False, debug=False)
    x_d = nc.dram_tensor("x", [rows_per_core, D], F32, kind="ExternalInput")
    w1h_d = nc.dram_tensor("w1h", [D, H], F16, kind="ExternalInput")
    w1l_d = nc.dram_tensor("w1l", [D, H], F16, kind="ExternalInput")
    w2h_d = nc.dram_tensor("w2h", [H, C], F16, kind="ExternalInput")
    w2l_d = nc.dram_tensor("w2l", [H, C], F16, kind="ExternalInput")
    dw1h_d = nc.dram_tensor("dw1h", [P, H], F16, kind="ExternalInput")
    dw1l_d = nc.dram_tensor("dw1l", [P, H], F16, kind="ExternalInput")
    dw2h_d = nc.dram_tensor("dw2h", [H, D], F16, kind="ExternalInput")
    dw2l_d = nc.dram_tensor("dw2l", [H, D], F16, kind="ExternalInput")
    b1_d = nc.dram_tensor("b1", [P, MH], F32, kind="ExternalInput")
    b2_d = nc.dram_tensor("b2", [P, MC], F32, kind="ExternalInput")
    db1_d = nc.dram_tensor("db1", [P, MH], F32, kind="ExternalInput")
    db2_d = nc.dram_tensor("db2", [P, D], F32, kind="ExternalInput")
    comp_d = nc.dram_tensor("comp", [C, 3], F32, kind="ExternalInput")
    compb_d = nc.dram_tensor("compb", [P, C * 3], F32, kind="ExternalInput")
    enc_d = nc.dram_tensor("enc", [rows_per_core, C * 3], F32, kind="ExternalOutput")
    dec_d = nc.dram_tensor("dec", [rows_per_core, D], F32, kind="ExternalOutput")

    with tile.TileContext(nc) as tc, ExitStack() as ctx:
        const = ctx.enter_context(tc.tile_pool(name="const", bufs=1))
        xpool = ctx.enter_context(tc.tile_pool(name="xp", bufs=4))
        xtp = ctx.enter_context(tc.tile_pool(name="xtp", bufs=3))
        xsp = ctx.enter_context(tc.tile_pool(name="xsp", bufs=KD))
        hsp = ctx.enter_context(tc.tile_pool(name="hsp", bufs=MH))
        dsp = ctx.enter_context(tc.tile_pool(name="dsp", bufs=MH))
        w1s = ctx.enter_context(tc.tile_pool(name="w1s", bufs=4))
        w2rs = ctx.enter_context(tc.tile_pool(name="w2rs", bufs=3))
        w2s = ctx.enter_context(tc.tile_pool(name="w2s", bufs=5))
        tk = ctx.enter_context(tc.tile_pool(name="tk", bufs=2))
        outp = ctx.enter_context(tc.tile_pool(name="outp", bufs=2))
        pacc = ctx.enter_context(tc.tile_pool(name="pacc", bufs=6, space="PSUM"))
        ptr = ctx.enter_context(tc.tile_pool(name="ptr", bufs=2, space="PSUM"))

        ident = const.tile([P, P], F32, name="ident", tag="ident")
        make_identity(nc, ident[:])
        tbl = const.tile([P, 1], F32, name="tbl", tag="tbl")
        nc.gpsimd.memset(tbl[:], 0.0)

        b1_sb = const.tile([P, MH], F32, name="b1", tag="b1")
        nc.sync.dma_start(b1_sb[:], b1_d[:])
        b2_sb = const.tile([P, MC], F32, name="b2", tag="b2")
        nc.sync.dma_start(b2_sb[:], b2_d[:])
        db1_sb = const.tile([P, MH], F32, name="db1", tag="db1")
        nc.sync.dma_start(db1_sb[:], db1_d[:])
        db2_sb = const.tile([P, D], F32, name="db2", tag="db2")
        nc.sync.dma_start(db2_sb[:], db2_d[:])
        compb_sb = const.tile([P, C * 3], F32, name="compb", tag="compb")
        nc.sync.dma_start(compb_sb[:], compb_d[:])
        dw1h_sb = const.tile([P, H], F16, name="dw1h", tag="dw1h")
        nc.sync.dma_start(dw1h_sb[:], dw1h_d[:])
        dw1l_sb = const.tile([P, H], F16, name="dw1l", tag="dw1l")
        nc.sync.dma_start(dw1l_sb[:], dw1l_d[:])
        comp_sb = []
        for kc in range(MC):
            t = const.tile([P, 3], F32, name=f"comp{kc}", tag=f"comp{kc}")
            nc.sync.dma_start(t[:], comp_d[kc * P:(kc + 1) * P, :])
            comp_sb.append(t)

        for blk in range(nblk):
            r0 = blk * BLK

            # ---- transpose x block into xT, split into fp16 hi/lo/scaled ----
            xts = []
            for kc in range(KD):
                xh = xsp.tile([P, BLK], F16, name="xh", tag="xh")
                xl = xsp.tile([P, BLK], F16, name="xl", tag="xl")
                xs = xsp.tile([P, BLK], F16, name="xs", tag="xs")
                xts.append((xh, xl, xs))
            for q in range(4):
                xrows = []
                for rt in range(rt_per_blk):
                    xrow = xpool.tile([P, 512], F32, name="x", tag="x")
                    nc.sync.dma_start(
                        xrow[:], x_d[r0 + rt * P: r0 + (rt + 1) * P,
                                     q * 512:(q + 1) * 512])
                    xrows.append(xrow)
                for rt in range(rt_per_blk):
                    xrow = xrows[rt]
                    for j in range(4):
                        kc = q * 4 + j
                        trp = ptr.tile([P, P], F32, name="tr", tag="tr")
                        nc.tensor.transpose(
                            trp[:], xrow[:, j * P:(j + 1) * P], ident[:])
                        xt32 = xtp.tile([P, P], F32, name="xt32", tag="s32x")
                        if kc % 2 == 0:
                            nc.vector.tensor_copy(xt32[:], trp[:])
                        else:
                            nc.scalar.copy(xt32[:], trp[:])
                        xh, xl, xs = xts[kc]
                        cs = slice(rt * P, (rt + 1) * P)
                        nc.vector.tensor_copy(xh[:, cs], xt32[:])
                        nc.vector.tensor_tensor(
                            xl[:, cs], xt32[:], xh[:, cs], op=ALU.subtract)
                        nc.scalar.activation(
                            xs[:, cs], xt32[:], AF.Copy, scale=1.0 / LS)

            # ---- enc1 (fp16x3): hT[m] = gelu(W1[:,m].T @ xT + b1_eff[m]) ----
            hts = [None] * MH
            for mh in range(2):
                accs = [pacc.tile([P, BLK], F32, name="acc", tag="acc")
                        for _ in range(MH // 2)]
                for k in range(KD):
                    w1ht = w1s.tile([P, HH], F16, name="w1ht", tag="w1ht")
                    nc.sync.dma_start(
                        w1ht[:], w1h_d[k * P:(k + 1) * P, mh * HH:(mh + 1) * HH])
                    w1lt = w1s.tile([P, HH], F16, name="w1lt", tag="w1lt")
                    nc.sync.dma_start(
                        w1lt[:], w1l_d[k * P:(k + 1) * P, mh * HH:(mh + 1) * HH])
                    xh, xl, xs = xts[k]
                    for m6 in range(MH // 2):
                        ws = slice(m6 * P, (m6 + 1) * P)
                        nc.tensor.matmul(accs[m6][:], w1ht[:, ws], xh[:],
                                         start=(k == 0), stop=False)
                        nc.tensor.matmul(accs[m6][:], w1ht[:, ws], xl[:],
                                         start=False, stop=False)
                        nc.tensor.matmul(accs[m6][:], w1lt[:, ws], xs[:],
                                         start=False, stop=(k == KD - 1))
                for m6 in range(MH // 2):
                    m = mh * (MH // 2) + m6
                    ht32 = xtp.tile([P, BLK], F32, name="ht32", tag="s32")
                    nc.scalar.activation(ht32[:], accs[m6][:], AF.Gelu,
                                         bias=b1_sb[:, m:m + 1])
                    hh = hsp.tile([P, BLK], F16, name="hh", tag="hh")
                    nc.vector.tensor_copy(hh[:], ht32[:])
                    hl = hsp.tile([P, BLK], F16, name="hl", tag="hl")
                    nc.vector.tensor_tensor(hl[:], ht32[:], hh[:],
                                            op=ALU.subtract)
                    hs = hsp.tile([P, BLK], F16, name="hs", tag="hs")
                    nc.scalar.activation(hs[:], ht32[:], AF.Copy,
                                         scale=1.0 / LS)
                    hts[m] = (hh, hl, hs)

            # preload the exp table set off the top-k critical chain
            nc.scalar.activation(tbl[:], tbl[:], AF.Exp)

            # ---- enc2 (fp16x3): logitsT[mc] = W2[:,mc].T @ hT + b2 ----
            lgaccs = [pacc.tile([P, BLK], F32, name="lga", tag="acc")
                      for _ in range(MC)]
            for k in range(MH):
                w2ht = w2rs.tile([P, C], F16, name="w2ht2", tag="w2ht2")
                nc.sync.dma_start(w2ht[:], w2h_d[k * P:(k + 1) * P, :])
                w2lt = w2rs.tile([P, C], F16, name="w2lt2", tag="w2lt2")
                nc.sync.dma_start(w2lt[:], w2l_d[k * P:(k + 1) * P, :])
                hh, hl, hs = hts[k]
                for mcc in range(MC):
                    ws = slice(mcc * P, (mcc + 1) * P)
                    nc.tensor.matmul(lgaccs[mcc][:], w2ht[:, ws], hh[:],
                                     start=(k == 0), stop=False)
                    nc.tensor.matmul(lgaccs[mcc][:], w2ht[:, ws], hl[:],
                                     start=False, stop=False)
                    nc.tensor.matmul(lgaccs[mcc][:], w2lt[:, ws], hs[:],
                                     start=False, stop=(k == MH - 1))
            lgts = []
            for mcc in range(MC):
                lt = tk.tile([P, BLK], F32, name=f"lgT{mcc}", tag=f"lgT{mcc}",
                             bufs=1)
                nc.scalar.activation(lt[:], lgaccs[mcc][:], AF.Identity,
                                     bias=b2_sb[:, mcc:mcc + 1])
                lgts.append(lt)

            # ---- top-4 masked softmax (3 passes so the PE pays the DVE
            # chain latency once, not per row-tile) ----
            pts_t = [tk.tile([P, BLK], F32, name=f"pT{mcc}", tag=f"pT{mcc}", bufs=1)
                     for mcc in range(MC)]
            lgs, ps = [], []
            for rt in range(rt_per_blk):
                lg = tk.tile([P, C], F32, name="lg", tag="lg", bufs=rt_per_blk)
                for mcc in range(MC):
                    trp = ptr.tile([P, P], F32, name="tr", tag="tr")
                    nc.tensor.transpose(
                        trp[:], lgts[mcc][:, rt * P:(rt + 1) * P], ident[:])
                    if mcc % 2 == 0:
                        nc.vector.tensor_copy(lg[:, mcc * P:(mcc + 1) * P], trp[:])
                    else:
                        nc.scalar.copy(lg[:, mcc * P:(mcc + 1) * P], trp[:])
                lgs.append(lg)
            for rt in range(rt_per_blk):
                lg = lgs[rt]
                t8 = tk.tile([P, 8], F32, name="t8", tag="t8")
                nc.vector.max(out=t8[:], in_=lg[:])
                e = tk.tile([P, C], F32, name="e", tag="e")
                nc.scalar.activation(e[:], lg[:], AF.Exp)
                # em = (lg >= t4) * e ; s = sum(em)
                em = tk.tile([P, C], F32, name="em", tag="em")
                s = tk.tile([P, 1], F32, name="s", tag="s")
                nc.vector.scalar_tensor_tensor(
                    em[:], lg[:], t8[:, 3:4], e[:],
                    op0=ALU.is_ge, op1=ALU.mult, accum_out=s[:])
                r = tk.tile([P, 1], F32, name="r", tag="r")
                nc.vector.reciprocal(r[:], s[:])
                p = tk.tile([P, C], F32, name="p", tag="p", bufs=rt_per_blk)
                nc.vector.tensor_scalar(p[:], em[:], r[:, 0:1], None, op0=ALU.mult)
                ps.append(p)
            for rt in range(rt_per_blk):
                p = ps[rt]
                for mcc in range(MC):
                    trp = ptr.tile([P, P], F32, name="tr", tag="tr")
                    nc.tensor.transpose(
                        trp[:], p[:, mcc * P:(mcc + 1) * P], ident[:])
                    if mcc % 2 == 0:
                        nc.vector.tensor_copy(
                            pts_t[mcc][:, rt * P:(rt + 1) * P], trp[:])
                    else:
                        nc.scalar.copy(
                            pts_t[mcc][:, rt * P:(rt + 1) * P], trp[:])

            # preload the gelu table set off the dec1 critical chain
            nc.scalar.activation(tbl[:], tbl[:], AF.Gelu)

            # ---- mix: ptsT = components.T @ pT, replicated at 4 row groups ----
            acc = pacc.tile([P, BLK], F32, name="acc", tag="acc")
            for g in range(4):
                for kc in range(MC):
                    nc.tensor.matmul(acc[g * 32:g * 32 + 3, :],
                                     comp_sb[kc][:, 0:3], pts_t[kc][:],
                                     start=(kc == 0), stop=(kc == MC - 1),
                                     tile_position=(0, g * 32))
            ptst = tk.tile([P, BLK], F32, name="ptsT", tag="ptsT", bufs=1)
            nc.scalar.copy(ptst[:], acc[:])
            psh = tk.tile([P, BLK], F16, name="psh", tag="psh", bufs=1)
            nc.vector.tensor_copy(psh[:], ptst[:])
            psl = tk.tile([P, BLK], F16, name="psl", tag="psl", bufs=1)
            nc.vector.tensor_tensor(psl[:], ptst[:], psh[:], op=ALU.subtract)
            pss = tk.tile([P, BLK], F16, name="pss", tag="pss", bufs=1)
            nc.scalar.activation(pss[:], ptst[:], AF.Copy, scale=1.0 / LS)

            # ---- dec1 (fp16x3, 4 row-groups packed): dT[m] = gelu(...) ----
            dsplit = []
            for mg in range(MH // 4):
                gaccs = [pacc.tile([P, BLK], F32, name="acc", tag="acc")
                         for _ in range(4)]
                for prod in range(3):
                    for g in range(4):
                        m = mg * 4 + g
                        ws = slice(m * P, (m + 1) * P)
                        gp = slice(g * 32, g * 32 + 3)
                        if prod == 0:
                            lhs, rhs = dw1h_sb[gp, ws], psh[gp, :]
                        elif prod == 1:
                            lhs, rhs = dw1h_sb[gp, ws], psl[gp, :]
                        else:
                            lhs, rhs = dw1l_sb[gp, ws], pss[gp, :]
                        nc.tensor.matmul(gaccs[g][:], lhs, rhs,
                                         start=(prod == 0), stop=(prod == 2),
                                         tile_position=(g * 32, 0))
                for g in range(4):
                    m = mg * 4 + g
                    dt32 = xtp.tile([P, BLK], F32, name="dt32", tag="s32")
                    nc.scalar.activation(dt32[:], gaccs[g][:], AF.Gelu,
                                         bias=db1_sb[:, m:m + 1])
                    dh = dsp.tile([P, BLK], F16, name="dh", tag="dh")
                    nc.vector.tensor_copy(dh[:], dt32[:])
                    dl = dsp.tile([P, BLK], F16, name="dl", tag="dl")
                    nc.vector.tensor_tensor(dl[:], dt32[:], dh[:],
                                            op=ALU.subtract)
                    ds = dsp.tile([P, BLK], F16, name="ds", tag="ds")
                    nc.scalar.activation(ds[:], dt32[:], AF.Copy, scale=1.0 / LS)
                    dsplit.append((dh, dl, ds))

            # ---- dec2 (fp16x3): decoded[b,n] = dT[:,b].T @ dw2[:,n] + db2 ----
            for n in range(ND):
                daccs = [pacc.tile([P, 512], F32, name="acc", tag="acc")
                         for _ in range(rt_per_blk)]
                for k in range(MH):
                    w2ht = w2s.tile([P, 512], F16, name="w2ht", tag="w2ht")
                    nc.sync.dma_start(
                        w2ht[:], dw2h_d[k * P:(k + 1) * P, n * 512:(n + 1) * 512])
                    w2lt = w2s.tile([P, 512], F16, name="w2lt", tag="w2lt")
                    nc.sync.dma_start(
                        w2lt[:], dw2l_d[k * P:(k + 1) * P, n * 512:(n + 1) * 512])
                    dh, dl, ds = dsplit[k]
                    for b in range(rt_per_blk):
                        bs = slice(b * P, (b + 1) * P)
                        nc.tensor.matmul(daccs[b][:], dh[:, bs], w2ht[:],
                                         start=(k == 0), stop=False)
                        nc.tensor.matmul(daccs[b][:], dl[:, bs], w2ht[:],
                                         start=False, stop=False)
                        nc.tensor.matmul(daccs[b][:], ds[:, bs], w2lt[:],
                                         start=False, stop=(k == MH - 1))
                for b in range(rt_per_blk):
                    o = outp.tile([P, 512], F32, name="out", tag="out")
                    nc.vector.tensor_add(o[:], daccs[b][:],
                                         db2_sb[:, n * 512:(n + 1) * 512])
                    nc.sync.dma_start(
                        dec_d[r0 + b * P: r0 + (b + 1) * P,
                              n * 512:(n + 1) * 512], o[:])

            # ---- encoded output: broadcast components ----
            for rt in range(rt_per_blk):
                nc.sync.dma_start(
                    enc_d[r0 + rt * P: r0 + (rt + 1) * P, :], compb_sb[:])

    nc.compile()
    return nc


_CACHE = {}


def _get_nc(rows_per_core: int):
    if rows_per_core not in _CACHE:
        _CACHE[rows_per_core] = _build(rows_per_core)
    return _CACHE[rows_per_core]


def _split16(w):
    hi = w.astype(np.float16)
    lo = ((w.astype(np.float64) - hi.astype(np.float64)) * LS).astype(np.float16)
    return np.ascontiguousarray(hi), np.ascontiguousarray(lo)


def prepare_inputs(x, layer_idx, embed, enc_w1, enc_b1, enc_w2, enc_b2,
                   components, dec_w1, dec_b1, dec_w2, dec_b2):
    x = np.ascontiguousarray(np.asarray(x, dtype=np.float32))
    embed = np.asarray(embed, dtype=np.float32)
    enc_w1 = np.ascontiguousarray(np.asarray(enc_w1, dtype=np.float32))
    enc_w2 = np.ascontiguousarray(np.asarray(enc_w2, dtype=np.float32))
    dec_w1 = np.ascontiguousarray(np.asarray(dec_w1, dtype=np.float32))
    dec_w2 = np.ascontiguousarray(np.asarray(dec_w2, dtype=np.float32))
    components = np.ascontiguousarray(np.asarray(components, dtype=np.float32))
    li = int(np.asarray(layer_idx))

    w1h, w1l = _split16(enc_w1)
    dw1h3, dw1l3 = _split16(dec_w1)
    # replicate dec_w1 at partition rows 0/32/64/96 for 4-way row-group packing
    dw1h = np.zeros((P, H), np.float16)
    dw1l = np.zeros((P, H), np.float16)
    for g in (0, 32, 64, 96):
        dw1h[g:g + 3] = dw1h3
        dw1l[g:g + 3] = dw1l3
    dw2h, dw2l = _split16(dec_w2)

    # Fold the embedding row into the encoder bias (in float64 on the host):
    # gelu((x+e) @ W1 + b1) == gelu(x @ W1 + (b1 + e @ W1)).
    b1_eff = (np.asarray(enc_b1, np.float64)
              + embed[li].astype(np.float64) @ enc_w1.astype(np.float64))
    w2h, w2l = _split16(enc_w2)
    shared = {
        "w1h": w1h, "w1l": w1l, "w2h": w2h, "w2l": w2l,
        "dw1h": dw1h, "dw1l": dw1l, "dw2h": dw2h, "dw2l": dw2l,
        "b1": np.ascontiguousarray(b1_eff.astype(np.float32).reshape(MH, P).T),
        "b2": np.ascontiguousarray(
            np.asarray(enc_b2, np.float32).reshape(MC, P).T),
        "db1": np.ascontiguousarray(
            np.asarray(dec_b1, np.float32).reshape(MH, P).T),
        "db2": np.ascontiguousarray(
            np.broadcast_to(np.asarray(dec_b2, np.float32), (P, D))),
        "comp": components,
        "compb": np.ascontiguousarray(
            np.broadcast_to(components.reshape(1, C * 3), (P, C * 3))),
    }
    return x, shared


def kernel(**inputs):
    x, shared = prepare_inputs(**inputs)
    nc = _get_nc(BC)
    in_maps = []
    for c in range(NCORES):
        m = dict(shared)
        m["x"] = np.ascontiguousarray(x[c * BC:(c + 1) * BC])
        in_maps.append(m)

    trace = bool(int(os.environ.get("KERNEL_TRACE", "0")))
    res = run_bass_kernel_spmd(nc, in_maps, core_ids=list(range(NCORES)),
                               trace=trace)
    if trace:
        kernel.last_exec_time_ns = res.exec_time_ns
        kernel.last_results = res

    decoded = np.concatenate([res.results[c]["dec"] for c in range(NCORES)], axis=0)
    encoded = np.concatenate([res.results[c]["enc"] for c in range(NCORES)],
                             axis=0).reshape(B, C, 3)
    return encoded, decoded
